# revision 13
# baseline (speedup 1.0000x reference)
"""Multi-head attention (raw-reshape variant) on 8 trn2 NeuronCores.

Shapes: B=2, S=2048, D=1024, H=16, dh=64.  The reference uses a raw
reshape (B,S,D)->(B,H,S,dh) (NOT a head transpose), so head h only sees
projected rows [128h, 128h+128).  Each (b, h) pair is therefore fully
independent: core c handles b=c//4 and the 4 heads of seq-block c%4.
No collectives; the host shards inputs and concatenates outputs.

Folded index convention per pair (128 input rows r, 1024 dims):
  s' = 16 r + t,  dm = 64 t + d   (t in [0,16), d in [0,64))
  Qfold[s', d] = Proj[r, 64 t + d]

Per-core plan:
  1. Qt/Kt assembly (fp32r): Qt[d, s'] as fp16 [128 x 2048] tiles with
     two pairs stacked in partition halves.  Direct matmuls out[d, r],
     M=64 col-packed across pair parity, N=256, contraction over dm_in.
     W and X chunks are host-concatenated so each k step is ONE DMA
     (the self-loading fp32r matmul can only encode one sync wait).
  2. V-proj (fp32r): natural Vproj[r, dm] -> Vaug fp16 [128 x 2048] with
     per-t blocks [V_t (64 cols) | ones (64 cols)]; the ones columns
     make the PV matmul emit 64 broadcast copies of the softmax
     denominator (M=128, full array, r for free).
  3. S^T per (pair, t, half) in fp16: St[r, q'] = sum_d Kt[d,16r+t] *
     Qt[d,q'], K=64 row-packed two pairs.  exp on ACT (scale=1/8 fused)
     -> fp16, mask multiply on DVE (fp16 2x mode).
  4. PV (fp16): psumO[128, 2048] += Vaug[:, 128t:+128].T @ P_t, K=128.
  5. normalize via DVE reciprocal + mul -> fp16 stack; pair B moved to
     partitions 64:127 with a small identity matmul so the final stage
     row-packs both pairs.
  6. final (fp16): out[r, :] += sum_t stack[:, t::16].T @ woT_blk[t],
     the inverse fold absorbed into PSUM accumulation.
"""

import numpy as np

import concourse.bass as bass
import concourse.mybir as mybir
import concourse.tile as tile
from concourse import bacc
from concourse.bass_utils import run_bass_kernel_spmd
from concourse.masks import make_identity

F32 = mybir.dt.float32
F16 = mybir.dt.float16
F32R = mybir.dt.float32r

B, S, D, H, DH = 2, 2048, 1024, 16, 64
N_CORES = 8
CORE_ROWS = 512          # seq rows per core
N_PAIRS = 4              # (b, h) pairs per core
EXP_SCALE = 0.125        # 1/sqrt(dh)

_NC = None


def _build_program():
    nc = bacc.Bacc()

    # host-concatenated [w_chunk | x_chunk] per contraction chunk k
    qasm = nc.dram_tensor("qasm", [8, 128, 1536], F16, kind="ExternalInput")
    kasm = nc.dram_tensor("kasm", [8, 128, 1536], F16, kind="ExternalInput")
    vasm = nc.dram_tensor("vasm", [8, 128, 1536], F32, kind="ExternalInput")
    woTblk = nc.dram_tensor("woTblk", [16, 64, D], F16, kind="ExternalInput")
    maskc_d = nc.dram_tensor("maskc", [S, S], F16, kind="ExternalInput")
    out_d = nc.dram_tensor("out", [CORE_ROWS, D], F32, kind="ExternalOutput")

    with tile.TileContext(nc) as tc:
        with tc.tile_pool(name="persist", bufs=1) as persist:
            maskc_sb = [persist.tile([128, S], F16, tag=f"mask{t}", name=f"mask{t}")
                        for t in range(16)]
            for t in range(16):
                nc.sync.dma_start(out=maskc_sb[t][:, :], in_=maskc_d[t::16, :])

            qt = [persist.tile([128, S], F16, tag=f"qt{g}", name=f"qt{g}") for g in range(2)]
            kt = [persist.tile([128, S], F16, tag=f"kt{g}", name=f"kt{g}") for g in range(2)]
            vaug = [persist.tile([128, 2048], F16, tag=f"vaug{p}", name=f"vaug{p}")
                    for p in range(N_PAIRS)]
            for p in range(N_PAIRS):
                va3 = vaug[p][:, :].rearrange("p (t c) -> p t c", c=128)
                nc.vector.memset(va3[:, :, 64:128], 1.0)
            stack = [persist.tile([128, S], F16, tag=f"stack{g}", name=f"stack{g}")
                     for g in range(2)]
            outsb = [persist.tile([128, D], F32, tag=f"outsb{p}", name=f"outsb{p}")
                     for p in range(N_PAIRS)]
            ident = persist.tile([64, 64], F16, tag="ident")
            make_identity(nc, ident[:, :])

            # ---------------- Phase 1: Qt/Kt assembly ----------------
            # PSUM start=True zeroes the mm's partition range across the
            # WHOLE bank, so every accumulation region must own its bank:
            # one [128 x 256] tile (bank-padded) per t, 8 banks per round.
            with tc.tile_pool(name="asm_mt", bufs=3) as mtpool:
              with tc.tile_pool(name="asm_ps", bufs=8, space="PSUM") as apsum:
                for ti, (src_d, dst) in enumerate(((qasm, qt), (kasm, kt))):
                    mts = []
                    for k in range(8):
                        mt = mtpool.tile([128, 1536], F16, tag="mtqk", name="mtqk",
                                         bufs=8)
                        nc.gpsimd.dma_start(out=mt[:, :], in_=src_d[k])
                        mts.append(mt)
                    for tgp in range(2):   # two rounds of 8 t-values
                        ps8 = [apsum.tile([128, 256], F32, tag="asm", name="asmps")
                               for _ in range(8)]
                        for k in range(8):
                            x3 = mts[k][:, 1024:1536].rearrange("p (i j) -> p i j", j=128)
                            for u in range(8):
                                t = 8 * tgp + u
                                for par in range(2):
                                    nc.tensor.matmul(
                                        ps8[u][64 * par:64 * (par + 1), :],
                                        lhsT=mts[k][:, 64 * t:64 * (t + 1)],
                                        rhs=x3[:, par::2, :],
                                        start=(k == 0), stop=(k == 7),
                                    )
                        # scatter: src col = g*128 + r ; dst col = t + 16 r
                        for u in range(8):
                            t = 8 * tgp + u
                            for g in range(2):
                                dst3 = dst[g][:, :].rearrange("p (r t) -> p t r", t=16)
                                nc.vector.tensor_copy(
                                    dst3[:, t, :],
                                    ps8[u][:, 128 * g:128 * (g + 1)])

              # ---------------- Phase 1b: V projection ----------------
              with tc.tile_pool(name="v_ps", bufs=4, space="PSUM") as vpsum:
                psv = [vpsum.tile([128, 1024], F32, tag="psv", name="psv")
                       for _ in range(N_PAIRS)]
                for k in range(8):
                    mt = mtpool.tile([128, 1536], F32R, tag="mtv", name="mtv")
                    nc.gpsimd.dma_start(out=mt[:, :], in_=vasm[k].bitcast(F32R))
                    for p in range(N_PAIRS):
                        for oc in range(2):
                            nc.tensor.matmul(
                                psv[p][:, 512 * oc:512 * (oc + 1)],
                                lhsT=mt[:, 1024 + 128 * p:1024 + 128 * (p + 1)],
                                rhs=mt[:, 512 * oc:512 * (oc + 1)],
                                start=(k == 0), stop=(k == 7),
                            )
                for p in range(N_PAIRS):
                    src = psv[p][:, :].rearrange("p (t c) -> p t c", c=64)
                    dst3 = vaug[p][:, :].rearrange("p (t c) -> p t c", c=128)
                    nc.vector.tensor_copy(dst3[:, :, 0:64], src)

            # ---------------- Phase 2: attention + output ----------------
            with tc.tile_pool(name="p_pool", bufs=3) as ppool, \
                 tc.tile_pool(name="norm", bufs=2) as npool, \
                 tc.tile_pool(name="wo_pool", bufs=3) as wopool, \
                 tc.tile_pool(name="st_ps", bufs=2, space="PSUM") as stpsum, \
                 tc.tile_pool(name="big_ps", bufs=1, space="PSUM") as bigpsum:
                for g in range(2):
                    for hp in range(2):
                        p = 2 * g + hp
                        lo, hi = 64 * hp, 64 * (hp + 1)
                        psO = bigpsum.tile([128, S], F32, tag="big", name="psO")
                        # software-pipelined: PV trails the St/exp/mask chain
                        work = []
                        for t in range(17):
                            if t < 16:
                                pms = []
                                for qh in range(2):
                                    stt = stpsum.tile([128, 1024], F32, tag="st", name="stt")
                                    for sc in range(2):
                                        nc.tensor.matmul(
                                            stt[:, 512 * sc:512 * (sc + 1)],
                                            lhsT=kt[g][lo:hi, t::16],
                                            rhs=qt[g][lo:hi, 1024 * qh + 512 * sc:1024 * qh + 512 * (sc + 1)],
                                            start=True, stop=True,
                                        )
                                    praw = ppool.tile([128, 1024], F16, tag="praw", name="praw")
                                    nc.scalar.activation(
                                        praw[:, :], stt[:, :],
                                        mybir.ActivationFunctionType.Exp,
                                        scale=EXP_SCALE,
                                    )
                                    pm = ppool.tile([128, 1024], F16, tag="pm", name="pm")
                                    nc.vector.tensor_mul(
                                        pm[:, :], praw[:, :],
                                        maskc_sb[t][:, 1024 * qh:1024 * (qh + 1)],
                                    )
                                    pms.append(pm)
                                work.append((t, pms))
                            if t >= 1:
                                tp, pms = work.pop(0)
                                for qh in range(2):
                                    for sc in range(2):
                                        nc.tensor.matmul(
                                            psO[:, 1024 * qh + 512 * sc:1024 * qh + 512 * (sc + 1)],
                                            lhsT=vaug[p][:, 128 * tp:128 * (tp + 1)],
                                            rhs=pms[qh][:, 512 * sc:512 * (sc + 1)],
                                            start=(tp == 0), stop=(tp == 15),
                                        )
                        # normalize: psO rows 64:128 hold the denominator
                        recip = npool.tile([64, S], F32, tag="recip", name="recip")
                        nc.vector.reciprocal(recip[:, :], psO[64:128, :])
                        if hp == 0:
                            nc.vector.tensor_mul(stack[g][0:64, :], psO[0:64, :], recip[:, :])
                        else:
                            tmpb = npool.tile([64, S], F16, tag="tmpb", name="tmpb")
                            nc.vector.tensor_mul(tmpb[:, :], psO[0:64, :], recip[:, :])
                            psZ = bigpsum.tile([128, S], F32, tag="big", name="psZ")
                            for zc in range(4):
                                nc.tensor.matmul(
                                    psZ[64:128, 512 * zc:512 * (zc + 1)],
                                    lhsT=ident[:, :],
                                    rhs=tmpb[:, 512 * zc:512 * (zc + 1)],
                                    start=True, stop=True,
                                )
                            nc.vector.tensor_copy(stack[g][64:128, :], psZ[64:128, :])

                    # final projection for group g, both pairs row-packed
                    psF = bigpsum.tile([128, S], F32, tag="big", name="psF")
                    for t in range(16):
                        wosb = wopool.tile([128, D], F16, tag="wo", name="wosb")
                        nc.gpsimd.dma_start(out=wosb[0:64, :], in_=woTblk[t])
                        nc.gpsimd.dma_start(out=wosb[64:128, :], in_=woTblk[t])
                        for hp in range(2):
                            lo, hi = 64 * hp, 64 * (hp + 1)
                            for oc in range(2):
                                nc.tensor.matmul(
                                    psF[:, 1024 * hp + 512 * oc:1024 * hp + 512 * (oc + 1)],
                                    lhsT=stack[g][lo:hi, t::16],
                                    rhs=wosb[lo:hi, 512 * oc:512 * (oc + 1)],
                                    start=(t == 0), stop=(t == 15),
                                )
                    for hp in range(2):
                        p = 2 * g + hp
                        nc.vector.tensor_copy(outsb[p][:, :], psF[:, 1024 * hp:1024 * (hp + 1)])
                        nc.gpsimd.dma_start(out=out_d[128 * p:128 * (p + 1), :], in_=outsb[p][:, :])

    nc.finalize()
    return nc


def build_in_maps(inputs):
    q = np.asarray(inputs["q"], dtype=np.float32)
    k = np.asarray(inputs["k"], dtype=np.float32)
    v = np.asarray(inputs["v"], dtype=np.float32)
    mask = np.asarray(inputs["mask"])
    w_q = np.asarray(inputs["w_q"], dtype=np.float32)
    w_k = np.asarray(inputs["w_k"], dtype=np.float32)
    w_v = np.asarray(inputs["w_v"], dtype=np.float32)
    w_o = np.asarray(inputs["w_o"], dtype=np.float32)

    # [8, 128, 1024] chunk views of the transposed weights
    wqT = np.ascontiguousarray(w_q.T).astype(np.float16).reshape(8, 128, D)
    wkT = np.ascontiguousarray(w_k.T).astype(np.float16).reshape(8, 128, D)
    wvT = np.ascontiguousarray(w_v.T).reshape(8, 128, D)
    woTblk = np.ascontiguousarray(w_o.T.reshape(16, 64, D)).astype(np.float16)
    # St tiles have k' on rows / q' on columns, so the mask complement is
    # loaded transposed: maskc[k', q'] = 1 - mask[b][q', k']
    maskc = [np.ascontiguousarray((~mask[b]).T.astype(np.float16)) for b in range(B)]

    in_maps = []
    for c in range(N_CORES):
        b, sb = c // 4, c % 4
        rows = slice(CORE_ROWS * sb, CORE_ROWS * (sb + 1))
        xqT = np.ascontiguousarray(q[b, rows].T).astype(np.float16).reshape(8, 128, CORE_ROWS)
        xkT = np.ascontiguousarray(k[b, rows].T).astype(np.float16).reshape(8, 128, CORE_ROWS)
        xvT = np.ascontiguousarray(v[b, rows].T).reshape(8, 128, CORE_ROWS)
        in_maps.append({
            "qasm": np.concatenate([wqT, xqT], axis=2),
            "kasm": np.concatenate([wkT, xkT], axis=2),
            "vasm": np.concatenate([wvT, xvT], axis=2),
            "woTblk": woTblk,
            "maskc": maskc[b],
        })
    return in_maps


def kernel(q, k, v, mask, w_q, w_k, w_v, w_o):
    global _NC
    if _NC is None:
        _NC = _build_program()

    in_maps = build_in_maps(dict(q=q, k=k, v=v, mask=mask,
                                 w_q=w_q, w_k=w_k, w_v=w_v, w_o=w_o))
    res = run_bass_kernel_spmd(_NC, in_maps, list(range(N_CORES))).results

    out = np.empty((B, S, D), dtype=np.float32)
    for c in range(N_CORES):
        b, sb = c // 4, c % 4
        out[b, CORE_ROWS * sb:CORE_ROWS * (sb + 1)] = res[c]["out"]
    return out
